# revision 1
# baseline (speedup 1.0000x reference)
"""Trainium2 Bass kernel for nn_Conv2dModulation.

Math (per sample b):
    w0 = weight * c,  c = (cin*3*3)^-0.5
    w1[o,i,kh,kw] = w0[o,i,kh,kw] * y[b,i]
    d[o] = rsqrt(sum_{i,kh,kw} w1^2 + eps)
    out[b] = conv2d_SAME(X[b], w1 * d)

Device strategy (per core, 2 samples):
  - Modulation/demodulation folded into a per-sample [64,64] weight tile per
    conv tap: lhsT[t][(s,i), o] = c * wT[t,i,o] * y[s,i] * d[s,o].
    d computed on device: S_T[i,o] = sum_t wT[t,i,o]^2;
    dpre[s,o] = sum_i y[s,i]^2 * S_T[i,o] (one small fp32 matmul);
    d = 1/sqrt(c^2*dpre + eps).
  - Conv = 9 shifted matmuls (taps) accumulating into PSUM.  X rows for both
    samples live on partition halves (s*64+ci); a 2x2 tile_position grid
    (rows = sample, cols = pixel half) runs 4 K=64/M=64 fp32r matmuls
    concurrently in the PE array quadrants.
  - PSUM written straight back to HBM by DMA (no evacuation pass).
"""

import numpy as np

import concourse.bass as bass
import concourse.tile as tile
from concourse import bacc, mybir
from concourse.bass_utils import run_bass_kernel_spmd

F32 = mybir.dt.float32
F32R = mybir.dt.float32r

B, C, H, W, KS = 16, 64, 256, 256, 3
NCORES = 8
SPC = B // NCORES          # samples per core = 2
WP = W + 2                 # padded row width (zero col at each edge)
EPS = 1e-8
CKAIMING = float((C * KS * KS) ** -0.5)

R = 32                     # output rows per chunk
NCHUNK = H // R

# knobs for fallbacks / experiments
USE_COL_TILING = True      # 2x2 quadrant grid (samples x pixel halves)
XT_BUFS = 3
PSUM_BUFS = 3


def build_program(nc):
    """Declare IO + emit the Tile program.  Returns input/output names."""
    Xl = nc.dram_tensor("Xl", [SPC * C, H, W], F32R, kind="ExternalInput")
    yt = nc.dram_tensor("yt", [C, SPC], F32, kind="ExternalInput")
    yb = nc.dram_tensor("yb", [SPC * C, 1], F32, kind="ExternalInput")
    wT = nc.dram_tensor("wT", [KS * KS, C, C], F32, kind="ExternalInput")
    zz = nc.dram_tensor("zz", [2 * C, WP], F32R, kind="ExternalInput")
    out = nc.dram_tensor("out", [SPC * C, H, W], F32, kind="ExternalOutput")

    with tile.TileContext(nc) as tc:
        with (
            tc.tile_pool(name="wpool", bufs=1) as wpool,
            tc.tile_pool(name="xpool", bufs=XT_BUFS) as xpool,
            tc.tile_pool(name="opool", bufs=4) as opool,
            tc.tile_pool(name="dram", bufs=1, space="DRAM") as dpool,
            tc.tile_pool(name="psA", bufs=2, space="PSUM") as psA,
            tc.tile_pool(name="psB", bufs=2, space="PSUM") as psB,
            tc.tile_pool(name="psC", bufs=2, space="PSUM") as psC,
            tc.tile_pool(name="psD", bufs=2, space="PSUM") as psD,
        ):
            # ---- prologue: modulated weights ----
            # wt2[(s,i), t*64+o] = wT[t,i,o] replicated on both partition halves
            wt2 = wpool.tile([2 * C, KS * KS * C], F32)
            wT_ap = wT.ap().rearrange("t i o -> i t o")
            for s in range(SPC):
                nc.sync.dma_start(
                    wt2[s * C:(s + 1) * C, :].rearrange(
                        "p (t o) -> p t o", t=KS * KS),
                    wT_ap,
                )

            yts = wpool.tile([C, SPC], F32)
            nc.sync.dma_start(yts[:, :], yt.ap()[:, :])
            ybs = wpool.tile([2 * C, 1], F32)
            nc.sync.dma_start(ybs[:, :], yb.ap()[:, :])

            ysq = wpool.tile([C, SPC], F32)
            nc.vector.tensor_mul(ysq[:, :], yts[:, :], yts[:, :])

            wsq = wpool.tile([C, KS * KS * C], F32)
            nc.vector.tensor_mul(wsq[:, :], wt2[0:C, :], wt2[0:C, :])
            s_acc = wpool.tile([C, C], F32)
            nc.vector.tensor_add(s_acc[:, :], wsq[:, 0:C], wsq[:, C:2 * C])
            for t in range(2, KS * KS):
                nc.vector.tensor_add(
                    s_acc[:, :], s_acc[:, :], wsq[:, t * C:(t + 1) * C])

            # dpre[s, o] = sum_i ysq[i,s] * S_T[i,o]
            dpre = psA.tile([SPC, C], F32, name="dpre", tag="p00")
            nc.tensor.matmul(dpre[:, :], ysq[:, :], s_acc[:, :],
                             start=True, stop=True)
            eps_t = wpool.tile([SPC, 1], F32)
            nc.gpsimd.memset(eps_t[:, :], EPS)
            dsq = wpool.tile([SPC, C], F32)
            nc.scalar.activation(dsq[:, :], dpre[:, :],
                                 mybir.ActivationFunctionType.Sqrt,
                                 bias=eps_t[:, :], scale=CKAIMING * CKAIMING)
            drow = wpool.tile([SPC, C], F32)
            nc.vector.reciprocal(drow[:, :], dsq[:, :])

            # broadcast d over partitions via a DRAM roundtrip
            ddr = dpool.tile([SPC, C], F32)
            nc.sync.dma_start(ddr[:, :], drow[:, :])
            dfull = wpool.tile([2 * C, C], F32)
            for s in range(SPC):
                nc.sync.dma_start(
                    dfull[s * C:(s + 1) * C, :],
                    ddr[s:s + 1, :].broadcast_to([C, C]),
                )

            # wy = wt2 * y(per-partition) * c ; wmod = wy * d (bcast over taps)
            wy = wpool.tile([2 * C, KS * KS * C], F32)
            nc.vector.tensor_scalar(wy[:, :], wt2[:, :], ybs[:, 0:1],
                                    CKAIMING, mybir.AluOpType.mult,
                                    mybir.AluOpType.mult)
            wmod = wpool.tile([2 * C, KS * KS * C], F32R)
            nc.vector.tensor_tensor(
                wmod[:, :].rearrange("p (t o) -> p t o", t=KS * KS),
                wy[:, :].rearrange("p (t o) -> p t o", t=KS * KS),
                dfull[:, :].unsqueeze(1).broadcast_to([2 * C, KS * KS, C]),
                mybir.AluOpType.mult,
            )

            # ---- conv main loop ----
            for ci in range(NCHUNK):
                r0 = ci * R
                xt = xpool.tile([2 * C, (R + 2) * WP], F32R)
                xt3 = xt[:, :].rearrange("p (r w) -> p r w", w=WP)
                # zero pad columns (w=-1 and w=W edges)
                zzap = zz.ap()
                nc.sync.dma_start(xt3[:, :, 0:1],
                                  zzap[:, 0:R + 2].unsqueeze(2))
                nc.sync.dma_start(xt3[:, :, WP - 1:WP],
                                  zzap[:, 0:R + 2].unsqueeze(2))
                lo = max(r0 - 1, 0)
                hi = min(r0 + R, H - 1)
                dst = lo - (r0 - 1)
                if ci == 0:
                    nc.sync.dma_start(xt3[:, 0:1, :], zzap.unsqueeze(1))
                if ci == NCHUNK - 1:
                    nc.sync.dma_start(xt3[:, R + 1:R + 2, :], zzap.unsqueeze(1))
                nc.sync.dma_start(
                    xt3[:, dst:dst + (hi - lo + 1), 1:W + 1],
                    Xl.ap()[:, lo:hi + 1, :],
                )

                for rb in range(R // 4):
                    pools = [[psA, psB], [psC, psD]]
                    tags = [["p00", "p01"], ["p10", "p11"]]
                    ps = [[pools[s][q].tile([C, 512], F32,
                                            name=f"ps{s}{q}_{ci}_{rb}",
                                            tag=tags[s][q])
                           for q in range(2)] for s in range(SPC)]
                    for t in range(KS * KS):
                        dh, dw = t // KS - 1, t % KS - 1
                        for q in range(2):
                            for s in range(SPC):
                                lhsT = wmod[s * C:(s + 1) * C,
                                            t * C:(t + 1) * C]
                                lr = rb * 4 + 2 * q + dh + 1
                                co = dw + 1
                                rhs = xt3[s * C:(s + 1) * C,
                                          lr:lr + 2, co:co + W]
                                nc.tensor.matmul(
                                    ps[s][q][:, :],
                                    lhsT,
                                    rhs,
                                    start=(t == 0),
                                    stop=(t == KS * KS - 1),
                                    tile_position=(s * C, 0),
                                    skip_group_check=True,
                                )
                    # evacuate PSUM -> SBUF (DVE: s=0, ACT: s=1 with
                    # partition shift), then one full-width DMA per rb
                    ostage = opool.tile([2 * C, 2 * 512], F32)
                    for q in range(2):
                        nc.vector.tensor_copy(
                            ostage[0:C, q * 512:(q + 1) * 512], ps[0][q][:, :])
                        nc.scalar.copy(
                            ostage[C:2 * C, q * 512:(q + 1) * 512],
                            ps[1][q][:, :])
                    dstap = out.ap()[:, r0 + rb * 4:r0 + rb * 4 + 4, :]
                    dstap = dstap.rearrange("so (q j) w -> so q (j w)", q=2)
                    nc.sync.dma_start(
                        dstap,
                        ostage[:, :].rearrange("p (q jw) -> p q jw", q=2))

    return nc


_CACHED = {}


def _get_compiled():
    if "nc" not in _CACHED:
        nc = bacc.Bacc("TRN2", debug=False)
        build_program(nc)
        nc.compile()
        _CACHED["nc"] = nc
    return _CACHED["nc"]


def make_in_maps(X, y, weight):
    X = np.ascontiguousarray(X, dtype=np.float32)
    y = np.ascontiguousarray(y, dtype=np.float32)
    weight = np.ascontiguousarray(weight, dtype=np.float32)
    wT = np.ascontiguousarray(
        weight.transpose(2, 3, 1, 0).reshape(KS * KS, C, C))
    in_maps = []
    for c in range(NCORES):
        xs = X[c * SPC:(c + 1) * SPC]
        ys = y[c * SPC:(c + 1) * SPC]
        in_maps.append({
            "Xl": np.ascontiguousarray(xs.reshape(SPC * C, H, W)),
            "zz": np.zeros((2 * C, WP), np.float32),
            "yt": np.ascontiguousarray(ys.T.reshape(C, SPC)),
            "yb": np.ascontiguousarray(ys.reshape(SPC * C, 1)),
            "wT": wT,
        })
    return in_maps


def kernel(X, y, weight):
    nc = _get_compiled()
    in_maps = make_in_maps(X, y, weight)
    res = run_bass_kernel_spmd(nc, in_maps, core_ids=list(range(NCORES)))
    outs = [res.results[c]["out"].reshape(SPC, C, H, W)
            for c in range(NCORES)]
    return np.concatenate(outs, axis=0)



# revision 3
# speedup vs baseline: 1.8861x; 1.8861x over previous
"""Trainium2 Bass kernel for nn_Conv2dModulation.

Math (per sample b):
    w0 = weight * c,  c = (cin*3*3)^-0.5
    w1[o,i,kh,kw] = w0[o,i,kh,kw] * y[b,i]
    d[o] = rsqrt(sum_{i,kh,kw} w1^2 + eps)
    out[b] = conv2d_SAME(X[b], w1 * d)

Device strategy (per core, 2 samples):
  - Modulation/demodulation folded into a per-sample [64,64] weight tile per
    conv tap (computed on device in fp32, stored bf16).
  - Conv = 9 shifted matmuls (taps) accumulating into PSUM.  All four
    64x64 PE-array quadrants run concurrently: rows = sample (s),
    cols = output-row pair (q), via tile_position=(s*64, q*64).  The
    (s,q) matmul writes PSUM bank_s partitions [q*64:(q+1)*64].
  - X streamed in bf16 (converted host-side), output evacuated
    PSUM(fp32) -> SBUF bf16 on DVE (s=0) / ACT (s=1), one DMA per
    4-row group, bf16 back to HBM (host converts to fp32).
"""

import numpy as np
import ml_dtypes

import concourse.bass as bass
import concourse.tile as tile
from concourse import bacc, mybir
from concourse.bass_utils import run_bass_kernel_spmd

F32 = mybir.dt.float32
BF16 = mybir.dt.bfloat16
NPBF16 = ml_dtypes.bfloat16

B, C, H, W, KS = 16, 64, 256, 256, 3
NCORES = 8
SPC = B // NCORES          # samples per core = 2
WP = W + 2                 # padded row width (zero col at each edge)
EPS = 1e-8
CKAIMING = float((C * KS * KS) ** -0.5)

R = 32                     # output rows per chunk
NCHUNK = H // R

XT_BUFS = 3


def build_program(nc):
    """Declare IO + emit the Tile program.  Returns input/output names."""
    Xl = nc.dram_tensor("Xl", [SPC * C, H, W], BF16, kind="ExternalInput")
    yt = nc.dram_tensor("yt", [C, SPC], F32, kind="ExternalInput")
    yb = nc.dram_tensor("yb", [SPC * C, 1], F32, kind="ExternalInput")
    wT = nc.dram_tensor("wT", [KS * KS, C, C], F32, kind="ExternalInput")
    out = nc.dram_tensor("out", [SPC * C, H, W], BF16, kind="ExternalOutput")

    with tile.TileContext(nc) as tc:
        with (
            tc.tile_pool(name="wpool", bufs=1) as wpool,
            tc.tile_pool(name="xpool", bufs=XT_BUFS) as xpool,
            tc.tile_pool(name="opool", bufs=4) as opool,
            tc.tile_pool(name="dram", bufs=1, space="DRAM") as dpool,
            tc.tile_pool(name="psA", bufs=4, space="PSUM") as psA,
            tc.tile_pool(name="psB", bufs=4, space="PSUM") as psB,
        ):
            # ---- prologue: modulated weights ----
            # wt2[(s,i), t*64+o] = wT[t,i,o] replicated on both partition halves
            wt2 = wpool.tile([2 * C, KS * KS * C], F32)
            wT_ap = wT.ap().rearrange("t i o -> i t o")
            for s in range(SPC):
                nc.sync.dma_start(
                    wt2[s * C:(s + 1) * C, :].rearrange(
                        "p (t o) -> p t o", t=KS * KS),
                    wT_ap,
                )

            yts = wpool.tile([C, SPC], F32)
            nc.sync.dma_start(yts[:, :], yt.ap()[:, :])
            ybs = wpool.tile([2 * C, 1], F32)
            nc.sync.dma_start(ybs[:, :], yb.ap()[:, :])

            ysq = wpool.tile([C, SPC], F32)
            nc.vector.tensor_mul(ysq[:, :], yts[:, :], yts[:, :])

            wsq = wpool.tile([C, KS * KS * C], F32)
            nc.vector.tensor_mul(wsq[:, :], wt2[0:C, :], wt2[0:C, :])
            s_acc = wpool.tile([C, C], F32)
            nc.vector.tensor_add(s_acc[:, :], wsq[:, 0:C], wsq[:, C:2 * C])
            for t in range(2, KS * KS):
                nc.vector.tensor_add(
                    s_acc[:, :], s_acc[:, :], wsq[:, t * C:(t + 1) * C])

            # dpre[s, o] = sum_i ysq[i,s] * S_T[i,o]
            dpre = psA.tile([SPC, C], F32, name="dpre", tag="p00")
            nc.tensor.matmul(dpre[:, :], ysq[:, :], s_acc[:, :],
                             start=True, stop=True)
            eps_t = wpool.tile([SPC, 1], F32)
            nc.gpsimd.memset(eps_t[:, :], EPS)
            dsq = wpool.tile([SPC, C], F32)
            nc.scalar.activation(dsq[:, :], dpre[:, :],
                                 mybir.ActivationFunctionType.Sqrt,
                                 bias=eps_t[:, :], scale=CKAIMING * CKAIMING)
            drow = wpool.tile([SPC, C], F32)
            nc.vector.reciprocal(drow[:, :], dsq[:, :])

            # broadcast d over partitions via a DRAM roundtrip
            ddr = dpool.tile([SPC, C], F32)
            nc.sync.dma_start(ddr[:, :], drow[:, :])
            dfull = wpool.tile([2 * C, C], F32)
            for s in range(SPC):
                nc.sync.dma_start(
                    dfull[s * C:(s + 1) * C, :],
                    ddr[s:s + 1, :].broadcast_to([C, C]),
                )

            # wy = wt2 * y(per-partition) * c ; wmod = wy * d (bcast over taps)
            wy = wpool.tile([2 * C, KS * KS * C], F32)
            nc.vector.tensor_scalar(wy[:, :], wt2[:, :], ybs[:, 0:1],
                                    CKAIMING, mybir.AluOpType.mult,
                                    mybir.AluOpType.mult)
            wmod = wpool.tile([2 * C, KS * KS * C], F32)
            nc.vector.tensor_tensor(
                wmod[:, :].rearrange("p (t o) -> p t o", t=KS * KS),
                wy[:, :].rearrange("p (t o) -> p t o", t=KS * KS),
                dfull[:, :].unsqueeze(1).broadcast_to([2 * C, KS * KS, C]),
                mybir.AluOpType.mult,
            )
            wmod16 = wpool.tile([2 * C, KS * KS * C], BF16)
            nc.scalar.copy(wmod16[:, :], wmod[:, :])

            # ---- conv main loop ----
            for ci in range(NCHUNK):
                r0 = ci * R
                xt = xpool.tile([2 * C, (R + 2) * WP], BF16)
                xt3 = xt[:, :].rearrange("p (r w) -> p r w", w=WP)
                # zero pad columns (w=-1 and w=W edges)
                nc.gpsimd.memset(xt3[:, :, 0:1], 0.0)
                nc.gpsimd.memset(xt3[:, :, WP - 1:WP], 0.0)
                lo = max(r0 - 1, 0)
                hi = min(r0 + R, H - 1)
                dst = lo - (r0 - 1)
                if ci == 0:
                    nc.gpsimd.memset(xt3[:, 0:1, 1:W + 1], 0.0)
                if ci == NCHUNK - 1:
                    nc.gpsimd.memset(xt3[:, R + 1:R + 2, 1:W + 1], 0.0)
                nc.sync.dma_start(
                    xt3[:, dst:dst + (hi - lo + 1), 1:W + 1],
                    Xl.ap()[:, lo:hi + 1, :],
                )

                for rb in range(R // 4):
                    ps = [
                        psA.tile([2 * C, 512], F32,
                                 name=f"ps0_{ci}_{rb}", tag="p00"),
                        psB.tile([2 * C, 512], F32,
                                 name=f"ps1_{ci}_{rb}", tag="p10"),
                    ]
                    for t in range(KS * KS):
                        dh, dw = t // KS - 1, t % KS - 1
                        for s in range(SPC):
                            lhsT = wmod16[s * C:(s + 1) * C,
                                          t * C:(t + 1) * C]
                            for q in range(2):
                                lr = rb * 4 + 2 * q + dh + 1
                                co = dw + 1
                                rhs = xt3[s * C:(s + 1) * C,
                                          lr:lr + 2, co:co + W]
                                nc.tensor.matmul(
                                    ps[s][q * C:(q + 1) * C, :],
                                    lhsT,
                                    rhs,
                                    start=(t == 0),
                                    stop=(t == KS * KS - 1),
                                    tile_position=(s * C, q * C),
                                    skip_group_check=True,
                                )
                    # evacuate PSUM -> SBUF bf16 (DVE: s=0, ACT: s=1) with
                    # partition remap (q,o) -> (s,o), then one DMA per rb
                    ostage = opool.tile([2 * C, 2 * 512], BF16)
                    for q in range(2):
                        nc.vector.tensor_copy(
                            ostage[0:C, q * 512:(q + 1) * 512],
                            ps[0][q * C:(q + 1) * C, :])
                        nc.scalar.copy(
                            ostage[C:2 * C, q * 512:(q + 1) * 512],
                            ps[1][q * C:(q + 1) * C, :])
                    rr = r0 + rb * 4
                    dstap = out.ap()[:, rr:rr + 4, :].rearrange(
                        "so (q t) w -> so q (t w)", q=2)
                    nc.sync.dma_start(
                        dstap,
                        ostage[:, :].rearrange("p (q tw) -> p q tw", q=2))

    return nc


_CACHED = {}


def _get_compiled():
    if "nc" not in _CACHED:
        nc = bacc.Bacc("TRN2", debug=False)
        build_program(nc)
        nc.compile()
        _CACHED["nc"] = nc
    return _CACHED["nc"]


def make_in_maps(X, y, weight):
    X = np.ascontiguousarray(X, dtype=np.float32)
    y = np.ascontiguousarray(y, dtype=np.float32)
    weight = np.ascontiguousarray(weight, dtype=np.float32)
    Xb = X.astype(NPBF16)
    wT = np.ascontiguousarray(
        weight.transpose(2, 3, 1, 0).reshape(KS * KS, C, C))
    in_maps = []
    for c in range(NCORES):
        xs = Xb[c * SPC:(c + 1) * SPC]
        ys = y[c * SPC:(c + 1) * SPC]
        in_maps.append({
            "Xl": np.ascontiguousarray(xs.reshape(SPC * C, H, W)),
            "yt": np.ascontiguousarray(ys.T.reshape(C, SPC)),
            "yb": np.ascontiguousarray(ys.reshape(SPC * C, 1)),
            "wT": wT,
        })
    return in_maps


def kernel(X, y, weight):
    nc = _get_compiled()
    in_maps = make_in_maps(X, y, weight)
    res = run_bass_kernel_spmd(nc, in_maps, core_ids=list(range(NCORES)))
    outs = [res.results[c]["out"].astype(np.float32).reshape(SPC, C, H, W)
            for c in range(NCORES)]
    return np.concatenate(outs, axis=0)


# revision 9
# speedup vs baseline: 2.3163x; 1.2281x over previous
"""Trainium2 Bass kernel for nn_Conv2dModulation.

Math (per sample b):
    w0 = weight * c,  c = (cin*3*3)^-0.5
    w1[o,i,kh,kw] = w0[o,i,kh,kw] * y[b,i]
    d[o] = rsqrt(sum_{i,kh,kw} w1^2 + eps)
    out[b] = conv2d_SAME(X[b], w1 * d)

Device strategy (per core, 2 samples):
  - Modulation/demodulation folded into a per-sample [64,64] weight tile per
    conv tap (computed on device in fp32, stored bf16).  The per-(s,o)
    demod factor is broadcast across partitions with a tiny indicator
    matmul (no DRAM roundtrip); Kaiming const folded into the indicator.
  - X is zero-padded on host to [H+2, W+2] so each 32-row chunk is ONE
    fully contiguous DMA (17.5KB/partition) and no memsets are needed.
  - Conv = 9 shifted matmuls (taps) accumulating into PSUM.  All four
    64x64 PE-array quadrants run concurrently: rows = sample (s),
    cols = output-row pair (q), via tile_position=(s*64, q*64).  The
    (s,q) matmul writes PSUM bank_s partitions [q*64:(q+1)*64].
  - PSUM (fp32) evacuated to SBUF bf16 on DVE (s=0) / ACT (s=1) with a
    partition remap (q,o)->(s,o); one batched DMA per 2 row-groups
    (8 output rows) back to HBM in bf16 (host converts to fp32).
"""

import numpy as np
import ml_dtypes

import concourse.bass as bass
import concourse.tile as tile
from concourse import bacc, mybir
from concourse.bass_utils import run_bass_kernel_spmd

F32 = mybir.dt.float32
BF16 = mybir.dt.bfloat16
NPBF16 = ml_dtypes.bfloat16

B, C, H, W, KS = 16, 64, 256, 256, 3
NCORES = 8
SPC = B // NCORES          # samples per core = 2
WP = W + 2                 # padded row width
HP = H + 2                 # padded column height
EPS = 1e-8
CKAIMING = float((C * KS * KS) ** -0.5)

R = 32                     # output rows per chunk
NCHUNK = H // R

XT_BUFS = 3


def build_program(nc):
    Xl = nc.dram_tensor("Xl", [SPC * C, HP, WP], BF16, kind="ExternalInput")
    yt = nc.dram_tensor("yt", [C, SPC], F32, kind="ExternalInput")
    yb = nc.dram_tensor("yb", [SPC * C, 1], F32, kind="ExternalInput")
    wT = nc.dram_tensor("wT", [KS * KS, C, C], F32, kind="ExternalInput")
    indt = nc.dram_tensor("indt", [SPC, 2 * C], F32, kind="ExternalInput")
    out = nc.dram_tensor("out", [SPC * C, H, W], BF16, kind="ExternalOutput")

    with tile.TileContext(nc) as tc:
        with (
            tc.tile_pool(name="wpool", bufs=1) as wpool,
            tc.tile_pool(name="xpool", bufs=XT_BUFS) as xpool,
            tc.tile_pool(name="opool", bufs=3) as opool,
            tc.tile_pool(name="psA", bufs=4, space="PSUM") as psA,
            tc.tile_pool(name="psB", bufs=4, space="PSUM") as psB,
        ):
            # ---- prologue: modulated weights ----
            # constants first (no deps -> off critical path)
            eps_t = wpool.tile([SPC, 1], F32)
            nc.gpsimd.memset(eps_t[:, :], EPS)
            # indicator for partition-broadcast of d: ind[s, (s2,i)] =
            # c * (s==s2); folds the Kaiming constant into the broadcast.
            ind = wpool.tile([SPC, 2 * C], F32)
            nc.sync.dma_start(ind[:, :], indt.ap()[:, :])

            # wt2[(s,i), t*64+o] = wT[t,i,o] on both partition halves
            wt2 = wpool.tile([2 * C, KS * KS * C], F32)
            wT_ap = wT.ap().rearrange("t i o -> i t o")
            for s in range(SPC):
                nc.sync.dma_start(
                    wt2[s * C:(s + 1) * C, :].rearrange(
                        "p (t o) -> p t o", t=KS * KS),
                    wT_ap,
                )
            yts = wpool.tile([C, SPC], F32)
            nc.sync.dma_start(yts[:, :], yt.ap()[:, :])
            ybs = wpool.tile([2 * C, 1], F32)
            nc.sync.dma_start(ybs[:, :], yb.ap()[:, :])

            # wy = wt2 * y  (per-partition scalar; DVE, overlaps d-chain)
            wy = wpool.tile([2 * C, KS * KS * C], F32)
            nc.vector.tensor_scalar(wy[:, :], wt2[:, :], ybs[:, 0:1],
                                    None, mybir.AluOpType.mult)

            ysq = wpool.tile([C, SPC], F32)
            nc.vector.tensor_mul(ysq[:, :], yts[:, :], yts[:, :])
            wsq = wpool.tile([C, KS * KS * C], F32)
            nc.vector.tensor_mul(wsq[:, :], wt2[0:C, :], wt2[0:C, :])
            s_acc = wpool.tile([C, C], F32)
            nc.vector.tensor_add(s_acc[:, :], wsq[:, 0:C], wsq[:, C:2 * C])
            for t in range(2, KS * KS):
                nc.vector.tensor_add(
                    s_acc[:, :], s_acc[:, :], wsq[:, t * C:(t + 1) * C])

            # dpre[s, o] = sum_i ysq[i,s] * S_T[i,o]
            dpre = psA.tile([SPC, C], F32, name="dpre", tag="p00")
            nc.tensor.matmul(dpre[:, :], ysq[:, :], s_acc[:, :],
                             start=True, stop=True)
            # drow[s,o] = 1/sqrt(c^2*dpre + eps)   (ACT reads PSUM)
            dsq = wpool.tile([SPC, C], F32)
            nc.scalar.activation(dsq[:, :], dpre[:, :],
                                 mybir.ActivationFunctionType.Sqrt,
                                 bias=eps_t[:, :], scale=CKAIMING * CKAIMING)
            drow = wpool.tile([SPC, C], F32)
            nc.vector.reciprocal(drow[:, :], dsq[:, :])
            # dfull[(s,i), o] = c * drow[s, o]  via indicator matmul
            dfull = psB.tile([2 * C, C], F32, name="dfull", tag="p10")
            nc.tensor.matmul(dfull[:, :], ind[:, :], drow[:, :],
                             start=True, stop=True)

            # wmod16[(s,i), t*64+o] = wy * c*d  (DVE reads PSUM operand)
            wmod16 = wpool.tile([2 * C, KS * KS * C], BF16)
            nc.vector.tensor_tensor(
                wmod16[:, :].rearrange("p (t o) -> p t o", t=KS * KS),
                wy[:, :].rearrange("p (t o) -> p t o", t=KS * KS),
                dfull[:, :].unsqueeze(1).broadcast_to([2 * C, KS * KS, C]),
                mybir.AluOpType.mult,
            )

            # ---- conv main loop ----
            for ci in range(NCHUNK):
                r0 = ci * R
                xt = xpool.tile([2 * C, (R + 2) * WP], BF16)
                xt3 = xt[:, :].rearrange("p (r w) -> p r w", w=WP)
                # padded rows r0..r0+33 = input rows r0-1..r0+32
                nc.sync.dma_start(xt3[:, :, :], Xl.ap()[:, r0:r0 + R + 2, :])

                for rbp in range(R // 8):          # pairs of row-groups
                    ostage = opool.tile([2 * C, 2 * 1024], BF16)
                    for g in range(2):
                        rb = rbp * 2 + g
                        ps = [
                            psA.tile([2 * C, 512], F32,
                                     name=f"ps0_{ci}_{rb}", tag="p00"),
                            psB.tile([2 * C, 512], F32,
                                     name=f"ps1_{ci}_{rb}", tag="p10"),
                        ]
                        for t in range(KS * KS):
                            dh, dw = t // KS - 1, t % KS - 1
                            for s in range(SPC):
                                lhsT = wmod16[s * C:(s + 1) * C,
                                              t * C:(t + 1) * C]
                                for q in range(2):
                                    lr = rb * 4 + 2 * q + dh + 1
                                    co = dw + 1
                                    rhs = xt3[s * C:(s + 1) * C,
                                              lr:lr + 2, co:co + W]
                                    nc.tensor.matmul(
                                        ps[s][q * C:(q + 1) * C, :],
                                        lhsT,
                                        rhs,
                                        start=(t == 0),
                                        stop=(t == KS * KS - 1),
                                        tile_position=(s * C, q * C),
                                        skip_group_check=True,
                                    )
                        # evacuate PSUM -> SBUF bf16 with partition remap
                        # (q,o) -> (s,o); DVE: s=0, ACT: s=1
                        for q in range(2):
                            nc.vector.tensor_copy(
                                ostage[0:C,
                                       g * 1024 + q * 512:
                                       g * 1024 + (q + 1) * 512],
                                ps[0][q * C:(q + 1) * C, :])
                            nc.scalar.copy(
                                ostage[C:2 * C,
                                       g * 1024 + q * 512:
                                       g * 1024 + (q + 1) * 512],
                                ps[1][q * C:(q + 1) * C, :])
                    rr = r0 + rbp * 8
                    dstap = out.ap()[:, rr:rr + 8, :].rearrange(
                        "so (gq t) w -> so gq (t w)", gq=4)
                    nc.sync.dma_start(
                        dstap,
                        ostage[:, :].rearrange("p (gq tw) -> p gq tw", gq=4))

    return nc


_CACHED = {}


def _get_compiled():
    if "nc" not in _CACHED:
        nc = bacc.Bacc("TRN2", debug=False)
        build_program(nc)
        nc.compile()
        _CACHED["nc"] = nc
    return _CACHED["nc"]


def make_in_maps(X, y, weight):
    X = np.ascontiguousarray(X, dtype=np.float32)
    y = np.ascontiguousarray(y, dtype=np.float32)
    weight = np.ascontiguousarray(weight, dtype=np.float32)
    Xp = np.zeros((B, C, HP, WP), dtype=NPBF16)
    Xp[:, :, 1:H + 1, 1:W + 1] = X.astype(NPBF16)
    wT = np.ascontiguousarray(
        weight.transpose(2, 3, 1, 0).reshape(KS * KS, C, C))
    ind = np.zeros((SPC, 2 * C), dtype=np.float32)
    for s in range(SPC):
        ind[s, s * C:(s + 1) * C] = CKAIMING
    in_maps = []
    for c in range(NCORES):
        xs = Xp[c * SPC:(c + 1) * SPC]
        ys = y[c * SPC:(c + 1) * SPC]
        in_maps.append({
            "Xl": np.ascontiguousarray(xs.reshape(SPC * C, HP, WP)),
            "yt": np.ascontiguousarray(ys.T.reshape(C, SPC)),
            "yb": np.ascontiguousarray(ys.reshape(SPC * C, 1)),
            "wT": wT,
            "indt": ind,
        })
    return in_maps


def kernel(X, y, weight):
    nc = _get_compiled()
    in_maps = make_in_maps(X, y, weight)
    res = run_bass_kernel_spmd(nc, in_maps, core_ids=list(range(NCORES)))
    outs = [res.results[c]["out"].astype(np.float32).reshape(SPC, C, H, W)
            for c in range(NCORES)]
    return np.concatenate(outs, axis=0)
